# revision 24
# baseline (speedup 1.0000x reference)
"""ProbAttention (Informer ProbSparse attention) Trainium2 kernel.

Strategy (8 NeuronCores, B*H = 64 (b,h) pairs -> 8 pairs per core):

Launch 1 (SIEVE): per core, for each 128-query tile, dma_gather the 40
sampled key rows (pair-major bf16, 1KiB elements) and compute the sampled
QK dots on DVE (broadcast-mul + grouped reduce) -> approximate sparsity
measure M for every query of every pair.

Host glue: top-64 candidates per pair from sieve M (validated: true top-40
is always contained, slack >= 21 ranks), exact f32 recompute of M on the
candidates (10M MACs numpy), exact top-40 selection.

Launch 2 (ATTN): per core/pair, gather the 40 winning query rows, hi/lo
bf16-split matmuls against K^T for f32-faithful scores, softmax (ACT exp
with accumulated denominator), PE-transpose of the attention weights, and
attn @ V accumulation -> 40 context rows per pair.

Host: output = copy of v with context rows scattered at the winning query
positions. Precision vs f32 reference: absmax ~8e-4 (scale ~5), L2 rel
~1.3e-5.
"""

import numpy as np
import ml_dtypes
from contextlib import ExitStack

import concourse.bass as bass
import concourse.bacc as bacc
import concourse.tile as tile
import concourse.mybir as mybir
import concourse.masks as masks
from concourse.bass_utils import run_bass_kernel_spmd

BF16 = ml_dtypes.bfloat16
DT = mybir.dt

B, L, H, D = 4, 2048, 16, 64
S = 40          # samples per query
NPAIR = B * H   # 64
NCORES = 8
PPC = NPAIR // NCORES  # 8 pairs per core
QT = L // 128   # 16 query tiles
NC_CAND = 64    # refine candidates per pair
U = 40          # top-u queries replaced per pair
D8 = PPC * D    # 512: pair-major feature width


def _bf(x):
    return np.ascontiguousarray(x).astype(BF16)


# ---------------------------------------------------------------------------
# device program builders
# ---------------------------------------------------------------------------

def build_sieve(repeat=1):
    nc = bacc.Bacc("TRN2", target_bir_lowering=False, debug=False)
    k8 = nc.dram_tensor("k8", [L, D8], DT.bfloat16, kind="ExternalInput")
    q8 = nc.dram_tensor("q8", [L, D8], DT.bfloat16, kind="ExternalInput")
    gidx = nc.dram_tensor("gidx", [QT, 128, S * 128 // 16], DT.int16,
                          kind="ExternalInput")
    m_out = nc.dram_tensor("m_out", [QT, 128, PPC], DT.float32,
                           kind="ExternalOutput")

    with tile.TileContext(nc) as tc, ExitStack() as ctx:
        resp = ctx.enter_context(tc.tile_pool(name="res", bufs=1))
        gp = ctx.enter_context(tc.tile_pool(name="gath", bufs=3))
        sp = ctx.enter_context(tc.tile_pool(name="small", bufs=4))

        # resident: all gather-index lists, all queries, all M outputs
        idx_all = resp.tile([128, QT, S * 128 // 16], DT.int16)
        nc.sync.dma_start(idx_all[:], gidx[:, :, :].rearrange("t p c -> p t c"))
        q_all = resp.tile([128, QT, D8], DT.bfloat16)
        nc.sync.dma_start(q_all[:], q8[:, :].rearrange("(t p) d -> p t d", p=128))
        m_all = resp.tile([128, QT, PPC], DT.float32)

        for _rep in range(repeat):
          for qt in range(QT):
            g_t = gp.tile([128, S, D8], DT.bfloat16)
            # dma_gather caps at 1024 indices per call on this runtime; split
            # the 5120-row gather into 5 chunks of 1024 (8 samples x 128
            # queries each; the s-major index list stays contiguous).
            for c5 in range(5):
                nc.gpsimd.dma_gather(
                    g_t[:, 8 * c5:8 * (c5 + 1), :], k8[:, :],
                    idx_all[:, qt, 64 * c5:64 * (c5 + 1)],
                    num_idxs=1024, num_idxs_reg=1024, elem_size=D8,
                )
            # g_t *= q (broadcast over the sample dim)
            nc.vector.tensor_mul(
                g_t[:], g_t[:],
                q_all[:, qt, :].unsqueeze(1).broadcast_to([128, S, D8]))
            # per-(sample, pair) dots: reduce innermost D
            qk_t = sp.tile([128, S * PPC], DT.float32, tag="qk")
            nc.vector.reduce_sum(
                qk_t[:], g_t[:].rearrange("p s (g d) -> p (s g) d", g=PPC),
                axis=mybir.AxisListType.X)
            v3 = qk_t[:].rearrange("p (s g) -> p g s", g=PPC)
            mx = sp.tile([128, PPC], DT.float32, tag="mx")
            nc.vector.reduce_max(mx[:], v3, axis=mybir.AxisListType.X)
            sm = sp.tile([128, PPC], DT.float32, tag="sm")
            nc.vector.reduce_sum(sm[:], v3, axis=mybir.AxisListType.X)
            # m = (sm * -1/L) + mx, written straight into the resident tile
            nc.vector.scalar_tensor_tensor(
                m_all[:, qt, :], sm[:], -1.0 / L, mx[:],
                op0=mybir.AluOpType.mult, op1=mybir.AluOpType.add)
        nc.sync.dma_start(m_out[:, :, :].rearrange("t p g -> p t g"), m_all[:])
    return nc


def build_attn(repeat=1):
    nc = bacc.Bacc("TRN2", target_bir_lowering=False, debug=False)
    q8f = nc.dram_tensor("q8f", [L, D8], DT.float32, kind="ExternalInput")
    kthi = nc.dram_tensor("kthi", [PPC, D, L], DT.bfloat16, kind="ExternalInput")
    ktlo = nc.dram_tensor("ktlo", [PPC, D, L], DT.bfloat16, kind="ExternalInput")
    v8 = nc.dram_tensor("v8", [L, D8], DT.bfloat16, kind="ExternalInput")
    # one wrapped 1024-index list: entry g*128+p = sel[g][p]*PPC+g (pad: g)
    widx = nc.dram_tensor("widx", [128, 64], DT.int16, kind="ExternalInput")
    ctx_out = nc.dram_tensor("ctx_out", [PPC, U, D], DT.float32,
                             kind="ExternalOutput")

    with tile.TileContext(nc) as tc, ExitStack() as ctx:
        constp = ctx.enter_context(tc.tile_pool(name="const", bufs=1))
        vp = ctx.enter_context(tc.tile_pool(name="v", bufs=1))
        ktp = ctx.enter_context(tc.tile_pool(name="kt", bufs=2))
        qwp = ctx.enter_context(tc.tile_pool(name="qw", bufs=2))
        smallp = ctx.enter_context(tc.tile_pool(name="small", bufs=4))
        ep = ctx.enter_context(tc.tile_pool(name="e", bufs=2))
        psc = ctx.enter_context(tc.tile_pool(name="psc", bufs=1, space="PSUM"))
        ptr = ctx.enter_context(tc.tile_pool(name="ptr", bufs=1, space="PSUM"))
        petr = ctx.enter_context(tc.tile_pool(name="petr", bufs=1, space="PSUM"))
        pctx = ctx.enter_context(tc.tile_pool(name="pctx", bufs=2, space="PSUM"))

        ident = constp.tile([128, 128], DT.bfloat16)
        masks.make_identity(nc, ident[:])

        # resident V (all pairs), viewed as 16 chunks of [128, D8]
        v_t = vp.tile([128, QT, D8], DT.bfloat16)
        nc.sync.dma_start(v_t[:], v8[:, :].rearrange("(c p) d -> p c d", p=128))

        # gather all pairs' winning query rows in one call:
        # row index space = q8f viewed as [L*PPC, D] (row j*PPC+g)
        wi_t = qwp.tile([128, 64], DT.int16, tag="wi")
        nc.sync.dma_start(wi_t[:], widx[:, :])
        qw_all = qwp.tile([128, PPC, D], DT.float32, tag="qw")
        nc.gpsimd.dma_gather(
            qw_all[:], q8f[:, :].rearrange("l (g d) -> (l g) d", d=D), wi_t[:],
            num_idxs=PPC * 128, num_idxs_reg=PPC * 128, elem_size=D,
        )
        qhi_all = qwp.tile([128, PPC * D], DT.bfloat16, tag="qhi")
        nc.vector.tensor_copy(qhi_all[:], qw_all[:].rearrange("p g d -> p (g d)"))
        qlo32 = qwp.tile([128, PPC * D], DT.float32, tag="qlo32")
        nc.vector.tensor_sub(qlo32[:], qw_all[:].rearrange("p g d -> p (g d)"),
                             qhi_all[:])
        qlo_all = qwp.tile([128, PPC * D], DT.bfloat16, tag="qlo")
        nc.vector.tensor_copy(qlo_all[:], qlo32[:])
        # per-pair transpose -> [d, q] lhsT blocks at base partition 0
        qhiT = qwp.tile([D, PPC, 128], DT.bfloat16, tag="qhiT")
        qloT = qwp.tile([D, PPC, 128], DT.bfloat16, tag="qloT")
        for c in range(PPC):
            tp = ptr.tile([D, 128], DT.bfloat16, tag="qT")
            nc.tensor.transpose(tp[:], qhi_all[:, c * D:(c + 1) * D], ident[:])
            nc.scalar.copy(qhiT[:, c, :], tp[:])
            tp2 = ptr.tile([D, 128], DT.bfloat16, tag="qT")
            nc.tensor.transpose(tp2[:], qlo_all[:, c * D:(c + 1) * D], ident[:])
            nc.scalar.copy(qloT[:, c, :], tp2[:])

        for _rep in range(repeat):
          for g in range(PPC):
            lhs_hi = qhiT[:, g, :U]
            lhs_lo = qloT[:, g, :U]

            kthi_t = ktp.tile([D, L], DT.bfloat16, tag="kthi")
            nc.sync.dma_start(kthi_t[:], kthi[g])
            ktlo_t = ktp.tile([D, L], DT.bfloat16, tag="ktlo")
            nc.sync.dma_start(ktlo_t[:], ktlo[g])

            sc = psc.tile([U, L], DT.float32)
            for c in range(4):
                cs = slice(c * 512, (c + 1) * 512)
                nc.tensor.matmul(sc[:, cs], lhs_hi, kthi_t[:, cs],
                                 start=True, stop=False)
                nc.tensor.matmul(sc[:, cs], lhs_lo, kthi_t[:, cs],
                                 start=False, stop=False)
                nc.tensor.matmul(sc[:, cs], lhs_hi, ktlo_t[:, cs],
                                 start=False, stop=True)

            mx = smallp.tile([U, 1], DT.float32, tag="mx")
            nc.vector.reduce_max(mx[:], sc[:], axis=mybir.AxisListType.X)
            nbias = smallp.tile([U, 1], DT.float32, tag="nbias")
            nc.vector.tensor_scalar_mul(nbias[:], mx[:], -1.0 / 8.0)
            esb = ep.tile([U, L], DT.bfloat16, tag="esb")
            sume = smallp.tile([U, 1], DT.float32, tag="sume")
            nc.scalar.activation(
                esb[:], sc[:], mybir.ActivationFunctionType.Exp,
                bias=nbias[:], scale=1.0 / 8.0, accum_out=sume[:])

            eT_p = petr.tile([128, QT * U], DT.bfloat16, tag="eT")
            for t in range(QT):
                nc.tensor.transpose(
                    eT_p[:, t * U:(t + 1) * U],
                    esb[:, t * 128:(t + 1) * 128], ident[:U, :U])
            eT = ep.tile([128, QT * U], DT.bfloat16, tag="eTs")
            nc.scalar.copy(eT[:], eT_p[:])

            ctx_p = pctx.tile([U, D], DT.float32)
            for t in range(QT):
                nc.tensor.matmul(ctx_p[:], eT[:, t * U:(t + 1) * U],
                                 v_t[:, t, g * D:(g + 1) * D],
                                 start=(t == 0), stop=(t == QT - 1))
            rcp = smallp.tile([U, 1], DT.float32, tag="rcp")
            nc.vector.reciprocal(rcp[:], sume[:])
            ctx_sb = smallp.tile([U, D], DT.float32, tag="ctxsb")
            nc.vector.tensor_scalar_mul(ctx_sb[:], ctx_p[:], rcp[:])
            nc.sync.dma_start(ctx_out[g], ctx_sb[:])
    return nc


_CACHE = {}


def _programs():
    if "sieve" not in _CACHE:
        s = build_sieve(); s.finalize()
        a = build_attn(); a.finalize()
        _CACHE["sieve"] = s
        _CACHE["attn"] = a
    return _CACHE["sieve"], _CACHE["attn"]


# ---------------------------------------------------------------------------
# host-side helpers
# ---------------------------------------------------------------------------

def _wrap16(arr, pad_to=None):
    """Wrap a flat int list into the 16-partition dma_gather layout,
    replicated to 128 partitions: element i lives at [i % 16, i // 16]."""
    a = np.asarray(arr, np.int16)
    if pad_to is not None and a.size < pad_to:
        a = np.concatenate([a, np.full(pad_to - a.size, -1, np.int16)])
    assert a.size % 16 == 0
    w = a.reshape(-1, 16).T  # [16, n/16]
    return np.ascontiguousarray(np.tile(w, (8, 1)))


def kernel(q, k, v, sample_idx, attn_mask):
    q = np.ascontiguousarray(np.asarray(q, np.float32))
    k = np.ascontiguousarray(np.asarray(k, np.float32))
    v = np.ascontiguousarray(np.asarray(v, np.float32))
    sidx = np.asarray(sample_idx).astype(np.int64)
    assert q.shape == (B, L, H, D) and sidx.shape == (L, S)

    sieve_nc, attn_nc = _programs()

    # pair-major layouts [L, pair, D]
    kp = np.ascontiguousarray(k.transpose(1, 0, 2, 3).reshape(L, NPAIR, D))
    qp_ = np.ascontiguousarray(q.transpose(1, 0, 2, 3).reshape(L, NPAIR, D))
    vp_ = np.ascontiguousarray(v.transpose(1, 0, 2, 3).reshape(L, NPAIR, D))

    # shared gather index lists per query tile (s-major within tile)
    gidx_np = np.stack([
        _wrap16(sidx[qt * 128:(qt + 1) * 128, :].T.reshape(-1))
        for qt in range(QT)
    ])  # [QT, 128, 320]

    # ---- launch 1: sieve ----
    in_maps = []
    for c in range(NCORES):
        sl = slice(c * PPC, (c + 1) * PPC)
        in_maps.append(dict(
            k8=_bf(kp[:, sl, :].reshape(L, D8)),
            q8=_bf(qp_[:, sl, :].reshape(L, D8)),
            gidx=gidx_np,
        ))
    res1 = run_bass_kernel_spmd(sieve_nc, in_maps, core_ids=list(range(NCORES)))
    # M[pair, q]
    M = np.empty((NPAIR, L), np.float32)
    for c in range(NCORES):
        mo = res1.results[c]["m_out"]  # [QT, 128, PPC]
        M[c * PPC:(c + 1) * PPC] = mo.transpose(2, 0, 1).reshape(PPC, L)

    # ---- host: candidates + exact refine + exact top-40 ----
    cand = np.argpartition(-M, NC_CAND - 1, axis=-1)[:, :NC_CAND]  # [NPAIR, NC]
    khp = kp.transpose(1, 0, 2)  # [pair, L, D]
    qhp = qp_.transpose(1, 0, 2)
    sel = np.empty((NPAIR, U), np.int64)
    for p in range(NPAIR):
        c_ = cand[p]
        kc = khp[p][sidx[c_], :]                      # [NC, S, D]
        qkc = np.einsum('cd,csd->cs', qhp[p][c_], kc)
        Mc = qkc.max(-1) - qkc.sum(-1, dtype=np.float32) / L
        sel[p] = c_[np.argsort(-Mc, kind="stable")[:U]]

    # ---- launch 2: attention ----
    khi = kp.astype(BF16)
    klo = (kp - khi.astype(np.float32)).astype(BF16)
    in_maps2 = []
    for c in range(NCORES):
        sl = slice(c * PPC, (c + 1) * PPC)
        arr = np.empty(PPC * 128, np.int64)
        for g in range(PPC):
            arr[g * 128:g * 128 + U] = sel[c * PPC + g] * PPC + g
            arr[g * 128 + U:(g + 1) * 128] = g      # pad: pair g, row 0
        wl = _wrap16(arr)                            # [128, 64]
        in_maps2.append(dict(
            q8f=np.ascontiguousarray(qp_[:, sl, :].reshape(L, D8)),
            kthi=np.ascontiguousarray(khi[:, sl, :].transpose(1, 2, 0)),
            ktlo=np.ascontiguousarray(klo[:, sl, :].transpose(1, 2, 0)),
            v8=_bf(vp_[:, sl, :].reshape(L, D8)),
            widx=wl,
        ))
    res2 = run_bass_kernel_spmd(attn_nc, in_maps2, core_ids=list(range(NCORES)))

    # ---- host: assemble output = v with ctx rows scattered ----
    out = v.transpose(0, 2, 1, 3).copy()              # [B, H, L, D]
    outp = out.reshape(NPAIR, L, D)
    for c in range(NCORES):
        ctx_rows = res2.results[c]["ctx_out"]          # [PPC, U, D]
        for g in range(PPC):
            outp[c * PPC + g][sel[c * PPC + g]] = ctx_rows[g]
    return np.ascontiguousarray(out.transpose(0, 2, 1, 3))
